# Initial kernel scaffold
#
"""Trainium2 Bass kernel for a binarized MLP (BNN) trained-mode forward pass.

Computation (reference):
    h = sign(BN(x @ sign(W1).T)); h = sign(BN(h @ sign(W2).T));
    h = sign(BN(h @ sign(W3).T)); out = h @ sign(W4).T
with BatchNorm1d in training mode (batch stats over the full 16384 batch),
gamma = 1, beta = 0.

Numerical design (bit-exact vs the fp64 pipeline on this input family):
  * With gamma > 0 and beta == 0, sign(BN(y)) == sign(y - mean(y)) -- the
    variance never matters.
  * Hidden activations use a {0,1} encoding: h01 = (y - mu > 0).  By shift
    invariance the downstream algebra is unchanged -- for any layer,
    sign(y_next - mean(y_next)) == (z - mean(z) > 0) where z = h01 @ S.T --
    and the final layer is recovered exactly as out = 2*z4 - rowsum(S4).
    (Host-verified: no layer has y == mu exactly, so is_gt == sign here.)
    The {0,1} activation is a single VectorE tensor_scalar op (add bias,
    is_gt), ~3x cheaper than the ScalarE Sign pipeline it replaces, and
    its accum_out produces the batch rowsums for the next layer's stats.
  * Layers 2..4 operate on {0,1} activations and +-1 weights: fp8e4m3
    represents both exactly, products are exact, and accumulation stays in
    integer range far below 2^24, so those layers are EXACT.  fp8 DoubleRow
    runs the PE at ~2x bf16 speed.
  * Layer 1 splits 256*x into two fp16 terms (error ~2^-22 relative; the
    256 scale keeps the residual term normal in fp16).  Host-verified on
    the fixed input data: zero sign flips, and the minimum |y - mu| margin
    after the split (3.1e-6) exceeds the exact pipeline's own minimum
    margin (8.7e-7).  Two fp16 passes replace the previous three bf16
    passes: 13 matmul streams per output tile instead of 21.
  * Batch means for layers 2/3 derive from per-feature row sums of the
    {0,1} activations: mean(z) == (S @ rowsum01) / B, exact in integers.
    Only two 4 KB all-reduces are needed; both are hidden behind matmul
    work by staging early PSUM chunks to SBUF fp16 (exact for integers)
    and signing them after the all-reduce lands.
  * Layer-1's mean is computed on the host in fp64 (exact distributivity:
    mean(x @ S1.T) == S1 @ mean(x)), so layer 1 needs no collective.

Sharding: data-parallel over the batch (16384 / 8 = 2048 rows per core),
weights replicated. Activations live on-chip in [feature, batch] layout.
"""

import numpy as np
import ml_dtypes

BF16 = ml_dtypes.bfloat16
FP8 = ml_dtypes.float8_e4m3
FP16 = np.float16

N_CORES = 8
B_FULL = 16384
B_SHARD = B_FULL // N_CORES  # 2048
D_IN = 784
NK1 = 6  # full 128-row k-tiles per split plane (768 rows)
LFT = D_IN - NK1 * 128  # 16 leftover rows per plane
D = 1024
D_OUT = 10
NCHUNK = 512
N_T = B_SHARD // NCHUNK  # 4
M_T = D // 128  # 8
A_T = D // 256  # 4 DoubleRow blocks of 256 contraction rows
XSCALE = 256.0  # sign-invariant pre-scale keeping fp16 split residuals normal

_PROGRAM = None
LAST_RESULTS = None  # BassKernelResults of the most recent device run


def _build_program(debug_outputs=False):
    from concourse import bacc
    import concourse.tile as tile
    import concourse.mybir as mybir

    f32 = mybir.dt.float32
    bf16 = mybir.dt.bfloat16
    fp16 = mybir.dt.float16
    fp8 = mybir.dt.float8e4
    AF_IDENT = mybir.ActivationFunctionType.Identity
    AF_SIGN = mybir.ActivationFunctionType.Sign
    AX = mybir.AxisListType.X
    DR = mybir.MatmulPerfMode.DoubleRow
    ADD = mybir.AluOpType.add
    MUL = mybir.AluOpType.mult
    GT = mybir.AluOpType.is_gt

    nc = bacc.Bacc(
        "TRN2", target_bir_lowering=False, debug=False, num_devices=N_CORES
    )

    xs_d = nc.dram_tensor("xs", [2, NK1 * 128, B_SHARD], fp16, kind="ExternalInput").ap()
    xlf_d = nc.dram_tensor("xlf", [2 * LFT, B_SHARD], fp16, kind="ExternalInput").ap()
    s1t_d = nc.dram_tensor("s1t", [NK1 * 128, D], fp16, kind="ExternalInput").ap()
    s1lf_d = nc.dram_tensor("s1lf", [2 * LFT, D], fp16, kind="ExternalInput").ap()
    s2b_d = nc.dram_tensor("s2b", [D, D], bf16, kind="ExternalInput").ap()
    s3b_d = nc.dram_tensor("s3b", [D, D], bf16, kind="ExternalInput").ap()
    s2dr_d = nc.dram_tensor("s2dr", [A_T, 128, 2, D], fp8, kind="ExternalInput").ap()
    s3dr_d = nc.dram_tensor("s3dr", [A_T, 128, 2, D], fp8, kind="ExternalInput").ap()
    # last dim padded 10 -> 16: DoubleRow LDWEIGHTS needs the pair-axis
    # step to be 16-byte aligned
    s4dr_d = nc.dram_tensor(
        "s4dr", [A_T, 128, 2, 16], fp8, kind="ExternalInput"
    ).ap()
    mu1_d = nc.dram_tensor("mu1", [128, M_T], f32, kind="ExternalInput").ap()
    negc4_d = nc.dram_tensor("negc4", [16, 1], f32, kind="ExternalInput").ap()
    out_d = nc.dram_tensor("out", [D_OUT, B_SHARD], f32, kind="ExternalOutput").ap()
    dbg = {}
    if debug_outputs:
        for nm, shape, dt_ in [
            ("dbg_h1", [128, M_T, B_SHARD], fp8),
            ("dbg_h2", [128, M_T, B_SHARD], fp8),
            ("dbg_nmu2", [128, M_T], f32),
            ("dbg_nmu3", [128, M_T], f32),
        ]:
            dbg[nm] = nc.dram_tensor(nm, shape, dt_, kind="ExternalOutput").ap()

    with tile.TileContext(nc) as tc:
        with (
            tc.tile_pool(name="w", bufs=1) as wp,
            tc.tile_pool(name="xb", bufs=2) as xp,
            tc.tile_pool(name="h", bufs=1) as hp,
            tc.tile_pool(name="h3", bufs=4) as h3p_pool,
            tc.tile_pool(name="stg", bufs=30) as stp,
            tc.tile_pool(name="small", bufs=1) as sp,
            tc.tile_pool(name="ob", bufs=2) as op_,
            tc.tile_pool(name="yps", bufs=6, space="PSUM") as yp,
            tc.tile_pool(name="mups", bufs=1, space="PSUM") as mp,
            tc.tile_pool(name="l4ps", bufs=1, space="PSUM") as lp,
            tc.tile_pool(name="dram", bufs=1, space="DRAM") as dp,
        ):
            # ---- layer-1 weights + x(n=0), ordered so the PE can start as
            # soon as the first plane lands --------------------------------
            s1w = [
                wp.tile([128, D], fp16, tag=f"s1_{k}", name=f"s1w{k}")
                for k in range(NK1)
            ]
            nc.sync.dma_start(out=s1w[0][:], in_=s1t_d[0:128, :])
            xc = [None] * N_T
            xlfc = [None] * N_T

            def load_x(n, split_dmas):
                """One [128, 12, 512] tile holding both (split, k) planes of
                batch chunk n -- a single DMA (or 2 on the first chunk so the
                PE can start sooner) -- plus the [32, 512] leftover rows."""
                t = xp.tile(
                    [128, 2 * NK1, NCHUNK], fp16, tag="x", name=f"x_{n}"
                )
                nsl = slice(n * NCHUNK, (n + 1) * NCHUNK)
                if split_dmas:
                    for s in range(2):
                        nc.sync.dma_start(
                            out=t[:, s * NK1 : (s + 1) * NK1, :],
                            in_=xs_d[s].rearrange("(kt p) b -> p kt b", p=128)[
                                :, :, nsl
                            ],
                        )
                else:
                    nc.sync.dma_start(
                        out=t[:],
                        in_=xs_d.rearrange("s (kt p) b -> p (s kt) b", p=128)[
                            :, :, nsl
                        ],
                    )
                xc[n] = t
                tl = xp.tile([2 * LFT, NCHUNK], fp16, tag="xlf", name=f"xlf_{n}")
                nc.sync.dma_start(out=tl[:], in_=xlf_d[:, nsl])
                xlfc[n] = tl

            load_x(0, split_dmas=True)
            mu1 = sp.tile([128, M_T], f32, tag="mu1", name="mu1")
            nc.sync.dma_start(out=mu1[:], in_=mu1_d)
            for k in range(1, NK1):
                nc.sync.dma_start(out=s1w[k][:], in_=s1t_d[k * 128 : (k + 1) * 128, :])
            s1lf = wp.tile([2 * LFT, D], fp16, tag="s1lf", name="s1lf")
            nc.sync.dma_start(out=s1lf[:], in_=s1lf_d)
            negc4 = sp.tile([16, 1], f32, tag="negc4", name="negc4")
            nc.sync.dma_start(out=negc4[:], in_=negc4_d)
            warm = sp.tile([128, 1], f32, tag="warm", name="warm")
            nc.gpsimd.memset(warm[:], 1.0)
            nc.scalar.sign(warm[:], warm[:])

            # h pairs in fp8 {0,1} (DoubleRow rhs layout: plane j of pair a
            # is feature tile m = 2a + j)
            h1p = [
                hp.tile([128, 2, B_SHARD], fp8, tag=f"h1_{a}", name=f"h1_{a}")
                for a in range(A_T)
            ]
            h2p = [
                hp.tile([128, 2, B_SHARD], fp8, tag=f"h2_{a}", name=f"h2_{a}")
                for a in range(A_T)
            ]
            hacc1 = sp.tile([128, M_T * N_T], f32, tag="hacc1", name="hacc1")
            hacc2 = sp.tile([128, M_T * N_T], f32, tag="hacc2", name="hacc2")

            def gt01(dst_ap, src_ap, mu_ap, accum_ap=None, eng=None):
                """h01 = (z > mu) in one op.  The comparison sits in op0
                (the verifier only restricts the op1/reduce slot, which
                stays a legal `add 0`), so accum_out can produce the {0,1}
                rowsum for the next layer's stats in the same instruction.
                eng selects VectorE (default) or GpSimd (SBUF sources only)
                so a long sign drain can run on both engines in parallel."""
                (eng or nc.vector).tensor_scalar(
                    dst_ap, src_ap, mu_ap, 0.0, op0=GT, op1=ADD,
                    accum_out=accum_ap,
                )

            # ---- layer 1 ----------------------------------------------
            for n in range(N_T):
                nsl = slice(n * NCHUNK, (n + 1) * NCHUNK)
                if n > 0:
                    load_x(n, split_dmas=False)
                for m0 in range(0, M_T, 2):
                    # two feature tiles interleaved: consecutive matmuls hit
                    # different PSUM banks, so array drains overlap
                    pss = [
                        yp.tile([128, NCHUNK], f32, tag="y", name=f"ps1_{n}_{m0 + j}")
                        for j in range(2)
                    ]
                    idx = 0
                    for s in range(2):
                        for k in range(NK1):
                            for j in range(2):
                                msl = slice((m0 + j) * 128, (m0 + j + 1) * 128)
                                nc.tensor.matmul(
                                    pss[j][:],
                                    s1w[k][:, msl],
                                    xc[n][:, s * NK1 + k, :],
                                    start=(idx == 0),
                                    stop=False,
                                )
                            idx += 1
                    for j in range(2):
                        msl = slice((m0 + j) * 128, (m0 + j + 1) * 128)
                        nc.tensor.matmul(
                            pss[j][:],
                            s1lf[:, msl],
                            xlfc[n][:],
                            start=False,
                            stop=True,
                        )
                    for j in range(2):
                        m = m0 + j
                        c = m * N_T + n
                        gt01(
                            h1p[m // 2][:, m % 2, nsl],
                            pss[j][:],
                            mu1[:, m : m + 1],
                            hacc1[:, c : c + 1],
                        )

            # ---- later-layer weights (emitted after L1 so their DMAs don't
            # delay the x stream) ---------------------------------------
            def load_ktiles(dram_ap, tagp):
                ts = []
                for k in range(M_T):
                    t = wp.tile([128, D], bf16, tag=f"{tagp}_{k}", name=f"{tagp}{k}")
                    nc.sync.dma_start(
                        out=t[:], in_=dram_ap[k * 128 : (k + 1) * 128, :]
                    )
                    ts.append(t)
                return ts

            def load_dr(dram_ap, tagp, dout):
                ts = []
                for a in range(A_T):
                    t = wp.tile([128, 2, dout], fp8, tag=f"{tagp}_{a}", name=f"{tagp}{a}")
                    nc.sync.dma_start(out=t[:], in_=dram_ap[a])
                    ts.append(t)
                return ts

            s2b = load_ktiles(s2b_d, "s2b")
            s2dr = load_dr(s2dr_d, "s2dr", D)
            s3b = load_ktiles(s3b_d, "s3b")
            s3dr = load_dr(s3dr_d, "s3dr", D)
            s4dr = load_dr(s4dr_d, "s4dr", 16)

            if debug_outputs:
                for a in range(A_T):
                    for j in range(2):
                        nc.sync.dma_start(
                            out=dbg["dbg_h1"][:, 2 * a + j, :], in_=h1p[a][:, j, :]
                        )

            # ---- stats pipeline ---------------------------------------
            # Per-feature batch means for the next layer, computed from the
            # LOCAL {0,1} rowsums: mean(z) = (S @ rowsum01) / B, and by
            # linearity the all-reduce can run on the matvec OUTPUT -- so the
            # PE-side mean matmul needs nothing from the collective, and only
            # the is_gt bias waits for it. Everything stays exact integers.
            # Engine split: rowsum splits on GpSimd (idle, SBUF-only), PSUM
            # combine on VectorE, scale on GpSimd -- so no long-wait op sits
            # in front of later DVE/Scalar work in their FIFO queues.
            def stats(hacc, sbw, lname, do_scale=True):
                hs = sp.tile([128, M_T], f32, tag=f"hs{lname}", name=f"hs{lname}")
                nc.vector.reduce_sum(
                    hs[:], hacc[:].rearrange("p (m n) -> p m n", n=N_T), axis=AX
                )
                # exact split of local integer rowsums (<= 2048) into two
                # bf16 halves for the mean matmul
                hh = sp.tile([128, M_T, 2], bf16, tag=f"hh{lname}", name=f"hh{lname}")
                hhif = sp.tile(
                    [128, M_T], f32, tag=f"hhif{lname}", name=f"hhif{lname}"
                )
                nc.gpsimd.tensor_copy(hh[:, :, 0], hs[:])
                nc.gpsimd.tensor_copy(hhif[:], hh[:, :, 0])
                nc.gpsimd.tensor_sub(hh[:, :, 1], hs[:], hhif[:])
                pmu = mp.tile([128, M_T, 2], f32, tag="mu", name=f"pmu{lname}")
                for m in range(M_T):
                    msl = slice(m * 128, (m + 1) * 128)
                    for k in range(M_T):
                        nc.tensor.matmul(
                            pmu[:, m, :],
                            sbw[k][:, msl],
                            hh[:, k, :],
                            start=(k == 0),
                            stop=(k == M_T - 1),
                        )
                pml = sp.tile([128, M_T], f32, tag=f"pml{lname}", name=f"pml{lname}")
                # pairwise add of the bf16-half columns == reduce over the
                # innermost axis (one PSUM input; two would be rejected)
                nc.vector.reduce_sum(pml[:], pmu[:], axis=AX)
                ar_in = dp.tile([128, M_T], f32, tag=f"ari{lname}", name=f"ari{lname}")
                ar_out = dp.tile([128, M_T], f32, tag=f"aro{lname}", name=f"aro{lname}")
                nc.sync.dma_start(out=ar_in[:], in_=pml[:])
                nc.gpsimd.collective_compute(
                    "AllReduce",
                    mybir.AluOpType.add,
                    replica_groups=[list(range(N_CORES))],
                    ins=[ar_in[:].opt()],
                    outs=[ar_out[:].opt()],
                )
                asb = sp.tile([128, M_T], f32, tag=f"asb{lname}", name=f"asb{lname}")
                nc.sync.dma_start(out=asb[:], in_=ar_out[:])
                if not do_scale:
                    # consumer folds the 1/B scale into its compare
                    return asb
                mu = sp.tile([128, M_T], f32, tag=f"mu{lname}", name=f"mu{lname}")
                nc.vector.tensor_scalar_mul(mu[:], asb[:], 1.0 / B_FULL)
                return mu

            # ---- layer 2 (fp8 DoubleRow, n-major; first 30 chunks staged
            # to fp16 so the PE never waits for the stats round-trip) ----
            def lx_mms_pair(ldr, h_in, n, m0, tagp):
                """Two DoubleRow chunks (m0, m0+1) with interleaved groups."""
                nsl = slice(n * NCHUNK, (n + 1) * NCHUNK)
                pss = [
                    yp.tile([128, NCHUNK], f32, tag="y", name=f"ps{tagp}_{n}_{m0 + j}")
                    for j in range(2)
                ]
                for a in range(A_T):
                    for j in range(2):
                        msl = slice((m0 + j) * 128, (m0 + j + 1) * 128)
                        nc.tensor.matmul(
                            pss[j][:],
                            ldr[a][:, :, msl],
                            h_in[a][:, :, nsl],
                            start=(a == 0),
                            stop=(a == A_T - 1),
                            perf_mode=DR,
                        )
                return pss

            def sign2(src_ap, n, m, eng=None):
                nsl = slice(n * NCHUNK, (n + 1) * NCHUNK)
                c = m * N_T + n
                gt01(
                    h2p[m // 2][:, m % 2, nsl],
                    src_ap,
                    mu2[:, m : m + 1],
                    hacc2[:, c : c + 1],
                    eng=eng,
                )

            pairs = [(n, m0) for n in range(N_T) for m0 in range(0, M_T, 2)]
            N_STG2P = 15  # staged chunk-pairs (30 chunks)
            staged2 = []

            def stage2(n, m0):
                pss = lx_mms_pair(s2dr, h1p, n, m0, "2")
                for j in range(2):
                    st = stp.tile(
                        [128, NCHUNK], fp16, tag="stg", name=f"st2_{n}_{m0 + j}"
                    )
                    # all stage copies on ScalarE: it is otherwise idle, and
                    # keeping them off VectorE means the sign drain and the
                    # stats fixups never block a copy in the DVE FIFO
                    nc.scalar.copy(st[:], pss[j][:])
                    staged2.append((st, n, m0 + j))

            # hacc1 completes with layer 1's last sign, so the stats matvec
            # is ready almost immediately -- emit it one pair in
            stage2(*pairs[0])
            mu2 = stats(hacc1, s2b, "1")
            for n, m0 in pairs[1:N_STG2P]:
                stage2(n, m0)
            for i, (st, n, m) in enumerate(staged2):
                sign2(st[:], n, m)
            for n, m0 in pairs[N_STG2P:]:
                pss = lx_mms_pair(s2dr, h1p, n, m0, "2")
                for j in range(2):
                    sign2(pss[j][:], n, m0 + j)

            if debug_outputs:
                for a in range(A_T):
                    for j in range(2):
                        nc.sync.dma_start(
                            out=dbg["dbg_h2"][:, 2 * a + j, :], in_=h2p[a][:, j, :]
                        )

            # ---- layer 3 + layer 4 ------------------------------------
            h3c = {}  # n -> list of pair tiles

            def h3_tile(n, m):
                a = m // 2
                if n not in h3c:
                    h3c[n] = [None] * A_T
                if h3c[n][a] is None:
                    h3c[n][a] = h3p_pool.tile(
                        [128, 2, NCHUNK], fp8, tag=f"h3_{a}", name=f"h3_{a}_{n}"
                    )
                return h3c[n][a]

            def sign3(src_ap, n, m):
                # even chunks: {0,1} on VectorE (compare B*z > M3, exact);
                # odd chunks: +-1 via ScalarE Sign(z - mu3) -- both engines
                # drain in parallel, halving the serial tail
                t = h3_tile(n, m)
                if n % 2 == 0:
                    nc.vector.tensor_scalar(
                        t[:, m % 2, :], src_ap, float(B_FULL),
                        M3[:, m : m + 1], op0=MUL, op1=GT,
                    )
                else:
                    nc.scalar.activation(
                        t[:, m % 2, :], src_ap, AF_SIGN,
                        bias=negmu3[:, m : m + 1],
                    )

            def l4(n):
                nsl = slice(n * NCHUNK, (n + 1) * NCHUNK)
                p4 = lp.tile([16, NCHUNK], f32, tag="y4", name=f"p4_{n}")
                for a in range(A_T):
                    nc.tensor.matmul(
                        p4[:],
                        s4dr[a][:],
                        h3c[n][a][:],
                        start=(a == 0),
                        stop=(a == A_T - 1),
                        perf_mode=DR,
                    )
                oc = op_.tile([D_OUT, NCHUNK], f32, tag="oc", name=f"oc_{n}")
                if n % 2 == 0:
                    # {0,1}-encoded h3: out = 2*z4 - rowsum(S4)
                    nc.scalar.activation(
                        oc[:],
                        p4[0:D_OUT, :],
                        AF_IDENT,
                        bias=negc4[0:D_OUT, :],
                        scale=2.0,
                    )
                else:
                    # +-1-encoded h3: z4 is already the exact output
                    nc.vector.tensor_copy(oc[:], p4[0:D_OUT, :])
                nc.sync.dma_start(out=out_d[:, nsl], in_=oc[:])

            N_STG3P = 15  # staged chunk-pairs (30 chunks), ring shared with L2
            staged3 = []

            def stage3(n, m0):
                pss = lx_mms_pair(s3dr, h2p, n, m0, "3")
                for j in range(2):
                    st = stp.tile(
                        [128, NCHUNK], fp16, tag="stg", name=f"st3_{n}_{m0 + j}"
                    )
                    nc.scalar.copy(st[:], pss[j][:])
                    staged3.append((st, n, m0 + j))

            # stats2's matvec depends on the full sign2 drain (which ends
            # mid-layer-2 thanks to the in-op accumulation); emit it one
            # pair in so the collective launches as early as possible
            stage3(*pairs[0])
            M3 = stats(hacc2, s3b, "2", do_scale=False)
            negmu3 = sp.tile([128, M_T], f32, tag="negmu3", name="negmu3")
            nc.vector.tensor_scalar_mul(negmu3[:], M3[:], -(1.0 / B_FULL))
            for n, m0 in pairs[1:N_STG3P]:
                stage3(n, m0)
            # drain the staged chunks n-major, launching each chunk's L4 as
            # soon as its 8 feature tiles are signed, so the output tail
            # overlaps the drain instead of serializing after it
            done_m = {n: 0 for n in range(N_T)}

            def after_sign(n):
                done_m[n] += 1
                if done_m[n] == M_T:
                    l4(n)

            for st, n, m in staged3:
                sign3(st[:], n, m)
                after_sign(n)
            for n, m0 in pairs[N_STG3P:]:
                pss = lx_mms_pair(s3dr, h2p, n, m0, "3")
                for j in range(2):
                    sign3(pss[j][:], n, m0 + j)
                    after_sign(n)

    nc.compile()
    return nc


def _get_program():
    global _PROGRAM
    if _PROGRAM is None:
        _PROGRAM = _build_program()
    return _PROGRAM


def _split2_fp16(a32):
    """Split fp32 array into two fp16 terms summing to a32 (to ~2^-22 rel)."""
    a0 = a32.astype(FP16)
    r = a32 - a0.astype(np.float32)
    a1 = r.astype(FP16)
    return a0, a1


def _dr_layout(st, dout):
    """[D, dout] K-major sign matrix -> DoubleRow lhsT blocks
    [A_T, 128, 2, dout] with element (a, ki, j, q) = st[(2a+j)*128 + ki, q]."""
    return np.ascontiguousarray(
        st.reshape(A_T, 2, 128, dout).transpose(0, 2, 1, 3).astype(FP8)
    )


def _numpy_fallback(x, W1, g1, b1, W2, g2, b2, W3, g3, b3, W4):
    eps = np.float32(1e-5)

    def bn_sign(y, g, b):
        mu = y.mean(axis=0, dtype=np.float32)
        var = np.mean(np.square(y - mu), axis=0, dtype=np.float32)
        return np.sign(g * (y - mu) / np.sqrt(var + eps) + b).astype(np.float32)

    h = bn_sign(x @ np.sign(W1).T, g1, b1)
    h = bn_sign(h @ np.sign(W2).T, g2, b2)
    h = bn_sign(h @ np.sign(W3).T, g3, b3)
    return (h @ np.sign(W4).T).astype(np.float32)


def kernel(x, W1, g1, b1, W2, g2, b2, W3, g3, b3, W4):
    global LAST_RESULTS
    x = np.asarray(x, np.float32).reshape(-1, D_IN)
    args = [np.asarray(a, np.float32) for a in (W1, g1, b1, W2, g2, b2, W3, g3, b3, W4)]
    W1, g1, b1, W2, g2, b2, W3, g3, b3, W4 = args

    specializable = (
        x.shape == (B_FULL, D_IN)
        and all((g > 0).all() for g in (g1, g2, g3))
        and all((b == 0).all() for b in (b1, b2, b3))
    )
    if not specializable:
        return _numpy_fallback(x, W1, g1, b1, W2, g2, b2, W3, g3, b3, W4)

    from concourse.bass_utils import run_bass_kernel_spmd

    s1 = np.sign(W1)  # [1024, 784]
    s1t_full = np.ascontiguousarray(s1.T.astype(FP16))  # [784, 1024]
    s1t = s1t_full[: NK1 * 128]
    s1lf = np.concatenate([s1t_full[NK1 * 128 :]] * 2, axis=0)  # [32, 1024]
    s2t = np.ascontiguousarray(np.sign(W2).T)  # [in, out] f32
    s3t = np.ascontiguousarray(np.sign(W3).T)
    s4t = np.ascontiguousarray(np.sign(W4).T)

    xt = np.ascontiguousarray(x.T) * np.float32(XSCALE)  # [784, 16384]
    x0, x1 = _split2_fp16(xt)
    xs_full = np.stack([x0[: NK1 * 128], x1[: NK1 * 128]])  # [2, 768, B]
    xlf_full = np.concatenate(
        [x0[NK1 * 128 :], x1[NK1 * 128 :]], axis=0
    )  # [32, B]

    # layer-1 batch mean, computed on host in fp64 (scaled by XSCALE):
    # mean(x @ S1.T, axis=0) == (S1 @ sum(x, axis=0)) / B
    xsum = x.sum(axis=0, dtype=np.float64)  # [784]
    mu1 = (s1.astype(np.float64) @ xsum) * (XSCALE / float(B_FULL))  # [1024]
    mu1_in = np.ascontiguousarray(
        mu1.astype(np.float32).reshape(M_T, 128).T
    )  # [128, M_T], column m <-> features m*128 + p
    # out = 2*z4 - rowsum(S4) fixup for the {0,1} encoding
    negc4 = np.zeros((16, 1), np.float32)
    negc4[:D_OUT, 0] = -s4t.sum(axis=0, dtype=np.float64).astype(np.float32)

    common = {
        "s1t": s1t,
        "s1lf": np.ascontiguousarray(s1lf),
        "s2b": np.ascontiguousarray(s2t.astype(BF16)),
        "s3b": np.ascontiguousarray(s3t.astype(BF16)),
        "s2dr": _dr_layout(s2t, D),
        "s3dr": _dr_layout(s3t, D),
        "s4dr": _dr_layout(np.concatenate([s4t, np.zeros((D, 6), s4t.dtype)], axis=1), 16),
        "mu1": mu1_in,
        "negc4": negc4,
    }
    in_maps = []
    for c in range(N_CORES):
        sl = slice(c * B_SHARD, (c + 1) * B_SHARD)
        in_maps.append(
            {
                "xs": np.ascontiguousarray(xs_full[:, :, sl]),
                "xlf": np.ascontiguousarray(xlf_full[:, sl]),
                **common,
            }
        )

    nc = _get_program()
    LAST_RESULTS = run_bass_kernel_spmd(nc, in_maps, core_ids=list(range(N_CORES)))
    y = np.concatenate(
        [LAST_RESULTS.results[c]["out"] for c in range(N_CORES)], axis=1
    )  # [10, 16384]
    return np.ascontiguousarray(y.T).astype(np.float32)



# revision 33
# speedup vs baseline: 1.1196x; 1.1196x over previous
"""Trainium2 Bass kernel for a binarized MLP (BNN) trained-mode forward pass.

Computation (reference):
    h = sign(BN(x @ sign(W1).T)); h = sign(BN(h @ sign(W2).T));
    h = sign(BN(h @ sign(W3).T)); out = h @ sign(W4).T
with BatchNorm1d in training mode (batch stats over the full 16384 batch),
gamma = 1, beta = 0.

Numerical design (bit-exact vs the fp64 pipeline on this input family):
  * With gamma > 0 and beta == 0, sign(BN(y)) == sign(y - mean(y)) -- the
    variance never matters.
  * Hidden activations use a {0,1} encoding: h01 = (y - mu > 0).  By shift
    invariance the downstream algebra is unchanged -- for any layer,
    sign(y_next - mean(y_next)) == (z - mean(z) > 0) where z = h01 @ S.T --
    and the final layer is recovered exactly as out = 2*z4 - rowsum(S4).
    (Host-verified: no layer has y == mu exactly, so is_gt == sign here.)
    The {0,1} activation is a single VectorE tensor_scalar op (add bias,
    is_gt), ~3x cheaper than the ScalarE Sign pipeline it replaces, and
    its accum_out produces the batch rowsums for the next layer's stats.
  * Layers 2..4 operate on {0,1} activations and +-1 weights: fp8e4m3
    represents both exactly, products are exact, and accumulation stays in
    integer range far below 2^24, so those layers are EXACT.  fp8 DoubleRow
    runs the PE at ~2x bf16 speed.
  * Layer 1 splits 256*x into two fp16 terms (error ~2^-22 relative; the
    256 scale keeps the residual term normal in fp16).  Host-verified on
    the fixed input data: zero sign flips, and the minimum |y - mu| margin
    after the split (3.1e-6) exceeds the exact pipeline's own minimum
    margin (8.7e-7).  Two fp16 passes replace the previous three bf16
    passes: 13 matmul streams per output tile instead of 21.
  * Batch means for layers 2/3 derive from per-feature row sums of the
    {0,1} activations: mean(z) == (S @ rowsum01) / B, exact in integers.
    Only two 4 KB all-reduces are needed; both are hidden behind matmul
    work by staging early PSUM chunks to SBUF fp16 (exact for integers)
    and signing them after the all-reduce lands.
  * Layer-1's mean is computed on the host in fp64 (exact distributivity:
    mean(x @ S1.T) == S1 @ mean(x)), so layer 1 needs no collective.

Sharding: data-parallel over the batch (16384 / 8 = 2048 rows per core),
weights replicated. Activations live on-chip in [feature, batch] layout.

Schedule/latency engineering (trace-driven):
  * A dummy 1-element all-reduce at t~0 (all DMAs on the GpSimd SWDGE
    queue -- a sync/HWDGE readback would wedge the input stream behind
    the collective) warms the CC core, taking ~11us of collective
    firmware init off the critical path.
  * A burst of throwaway matmuls on memset data keeps the PE busy during
    the initial x DMA so the HAM clock-gate is at full rate when real
    work arrives; the first x chunk is split so plane 0 lands early.
  * The four K=32 leftover matmuls of each layer-1 group run
    concurrently in 32-row strips (tile_position row tiling) -- one
    512-cycle pass instead of four (weights/x leftovers replicated 4x
    along partitions on the host).
  * Layer-2 signs split by chunk parity: even chunks {0,1} on VectorE,
    odd chunks +-1 via ScalarE Sign, so the post-collective drain runs
    on both engines in parallel.  Odd-chunk z3 columns then carry true
    +-1-encoded values, compared against negmu3pm = c3 - (2/B)*M3
    (c3 = rowsum(S3), host-precomputed; exact in fp32), while even
    columns stay in the {0,1} algebra against M3.  The h2 rowsums are
    recovered exactly from the two encodings for the mu3 matvec.
  * All layer-2 chunks and the first 13 layer-3 pairs stage PSUM->SBUF
    fp16 (exact for these integers) so the matmul stream never waits
    on the collectives; the last 3 layer-3 pairs hold their PSUM banks
    and sign directly once mu3 lands.
"""

import numpy as np
import ml_dtypes

BF16 = ml_dtypes.bfloat16
FP8 = ml_dtypes.float8_e4m3
FP16 = np.float16

N_CORES = 8
B_FULL = 16384
B_SHARD = B_FULL // N_CORES  # 2048
D_IN = 784
NK1 = 6  # full 128-row k-tiles per split plane (768 rows)
LFT = D_IN - NK1 * 128  # 16 leftover rows per plane
D = 1024
D_OUT = 10
NCHUNK = 512
N_T = B_SHARD // NCHUNK  # 4
M_T = D // 128  # 8
A_T = D // 256  # 4 DoubleRow blocks of 256 contraction rows
XSCALE = 256.0  # sign-invariant pre-scale keeping fp16 split residuals normal

_PROGRAM = None
LAST_RESULTS = None  # BassKernelResults of the most recent device run


def _build_program(debug_outputs=False):
    from concourse import bacc
    import concourse.tile as tile
    import concourse.mybir as mybir

    f32 = mybir.dt.float32
    bf16 = mybir.dt.bfloat16
    fp16 = mybir.dt.float16
    fp8 = mybir.dt.float8e4
    AF_IDENT = mybir.ActivationFunctionType.Identity
    AF_SIGN = mybir.ActivationFunctionType.Sign
    AX = mybir.AxisListType.X
    DR = mybir.MatmulPerfMode.DoubleRow
    ADD = mybir.AluOpType.add
    MUL = mybir.AluOpType.mult
    GT = mybir.AluOpType.is_gt

    nc = bacc.Bacc(
        "TRN2", target_bir_lowering=False, debug=False, num_devices=N_CORES
    )

    xs_d = nc.dram_tensor("xs", [2, NK1 * 128, B_SHARD], fp16, kind="ExternalInput").ap()
    # leftover rows replicated 4x along partitions so four K=32 leftover
    # matmuls can run concurrently in 32-row strips via tile_position
    xlf_d = nc.dram_tensor("xlf", [4 * 2 * LFT, B_SHARD], fp16, kind="ExternalInput").ap()
    s1t_d = nc.dram_tensor("s1t", [NK1 * 128, D], fp16, kind="ExternalInput").ap()
    s1lf_d = nc.dram_tensor("s1lf", [4 * 2 * LFT, D], fp16, kind="ExternalInput").ap()
    s2b_d = nc.dram_tensor("s2b", [D, D], bf16, kind="ExternalInput").ap()
    s3b_d = nc.dram_tensor("s3b", [D, D], bf16, kind="ExternalInput").ap()
    s2dr_d = nc.dram_tensor("s2dr", [A_T, 128, 2, D], fp8, kind="ExternalInput").ap()
    s3dr_d = nc.dram_tensor("s3dr", [A_T, 128, 2, D], fp8, kind="ExternalInput").ap()
    # last dim padded 10 -> 16: DoubleRow LDWEIGHTS needs the pair-axis
    # step to be 16-byte aligned
    s4dr_d = nc.dram_tensor(
        "s4dr", [A_T, 128, 2, 16], fp8, kind="ExternalInput"
    ).ap()
    mu1_d = nc.dram_tensor("mu1", [128, M_T], f32, kind="ExternalInput").ap()
    c3_d = nc.dram_tensor("c3", [128, M_T], f32, kind="ExternalInput").ap()
    negc4_d = nc.dram_tensor("negc4", [16, 1], f32, kind="ExternalInput").ap()
    out_d = nc.dram_tensor("out", [D_OUT, B_SHARD], f32, kind="ExternalOutput").ap()
    dbg = {}
    if debug_outputs:
        for nm, shape, dt_ in [
            ("dbg_h1", [128, M_T, B_SHARD], fp8),
            ("dbg_h2", [128, M_T, B_SHARD], fp8),
            ("dbg_nmu2", [128, M_T], f32),
            ("dbg_nmu3", [128, M_T], f32),
        ]:
            dbg[nm] = nc.dram_tensor(nm, shape, dt_, kind="ExternalOutput").ap()

    with tile.TileContext(nc) as tc:
        with (
            tc.tile_pool(name="w", bufs=1) as wp,
            tc.tile_pool(name="xb", bufs=2) as xp,
            tc.tile_pool(name="h", bufs=1) as hp,
            tc.tile_pool(name="h3", bufs=4) as h3p_pool,
            tc.tile_pool(name="stg", bufs=32) as stp,
            tc.tile_pool(name="small", bufs=1) as sp,
            tc.tile_pool(name="ob", bufs=2) as op_,
            tc.tile_pool(name="yps", bufs=6, space="PSUM") as yp,
            # stats-matvec and layer-4 PSUM share one 2-slot tag: their
            # live ranges alternate (pmu1 ~130us, pmu2 ~185us, p4 210us+),
            # keeping the matmul stream at 6 banks
            tc.tile_pool(name="mups", bufs=2, space="PSUM") as mp,
            tc.tile_pool(name="dram", bufs=1, space="DRAM") as dp,
        ):
            # ---- cold-start amortization --------------------------------
            # (1) a dummy 1-element all-reduce issued at t~0 warms the CC
            # core path (the first collective otherwise pays ~11us of
            # firmware init on the critical path) and absorbs most of the
            # inter-core launch skew while the PE is still crunching L1.
            # (2) a burst of throwaway matmuls on memset data keeps the PE
            # busy during the initial x DMA so the HAM clock-gate reaches
            # 2.4 GHz before real work arrives (saves the 1.2 GHz ramp).
            wcc_in = dp.tile([1, 1], f32, tag="wcci", name="wcci")
            wcc_out = dp.tile([1, 1], f32, tag="wcco", name="wcco")
            # NOTE: every DMA here goes through the GpSimd SWDGE queue, NOT
            # nc.sync -- a sync (HWDGE) readback would sit in the same FIFO
            # as the x/weight loads and wedge the whole input stream behind
            # the collective (measured: 590us stall). GpSimd has no other
            # work until the layer-2 stats (~130us), so blocking its queue
            # on the dummy all-reduce is harmless.
            wtmp = sp.tile([1, 1], f32, tag="wtmp", name="wtmp")
            nc.gpsimd.memset(wtmp[:], 0.0)
            nc.gpsimd.dma_start(out=wcc_in[:], in_=wtmp[:])
            nc.gpsimd.collective_compute(
                "AllReduce",
                mybir.AluOpType.add,
                replica_groups=[list(range(N_CORES))],
                ins=[wcc_in[:].opt()],
                outs=[wcc_out[:].opt()],
            )
            wcc_sb = sp.tile([1, 1], f32, tag="wccsb", name="wccsb")
            nc.gpsimd.dma_start(out=wcc_sb[:], in_=wcc_out[:])
            nc.gpsimd.tensor_scalar_mul(wcc_sb[:], wcc_sb[:], 0.0)

            wlhs = sp.tile([128, 128], fp16, tag="wl", name="wl")
            wrhs = sp.tile([128, NCHUNK], fp16, tag="wr", name="wr")
            nc.gpsimd.memset(wlhs[:], 0.0)
            nc.gpsimd.memset(wrhs[:], 0.0)
            wps = yp.tile([128, NCHUNK], f32, tag="y", name="warmps")
            N_WARM_MM = 18  # one HAM window (~3.4us cold) + cover until the
            # second x k-plane DMA lands (~18us)
            for i in range(N_WARM_MM):
                nc.tensor.matmul(
                    wps[:], wlhs[:], wrhs[:],
                    start=(i == 0), stop=(i == N_WARM_MM - 1),
                )
            warmred = sp.tile([128, 1], f32, tag="warmred", name="warmred")
            nc.vector.reduce_sum(warmred[:], wps[:], axis=AX)
            # ---- layer-1 weights + x(n=0), ordered so the PE can start as
            # soon as the first plane lands --------------------------------
            s1w = [
                wp.tile([128, D], fp16, tag=f"s1_{k}", name=f"s1w{k}")
                for k in range(NK1)
            ]
            nc.sync.dma_start(out=s1w[0][:], in_=s1t_d[0:128, :])
            xc = [None] * N_T
            xlfc = [None] * N_T

            def load_x(n, split_dmas):
                """One [128, 12, 512] tile holding both (split, k) planes of
                batch chunk n -- a single DMA (or 2 on the first chunk so the
                PE can start sooner) -- plus the [32, 512] leftover rows."""
                t = xp.tile(
                    [128, 2 * NK1, NCHUNK], fp16, tag="x", name=f"x_{n}"
                )
                nsl = slice(n * NCHUNK, (n + 1) * NCHUNK)
                if split_dmas:
                    # first k-plane alone (lands ~5us earlier, PE starts on
                    # it), then the rest of split 0, then split 1
                    for lo, hi, s in ((0, 1, 0), (1, NK1, 0), (0, NK1, 1)):
                        nc.sync.dma_start(
                            out=t[:, s * NK1 + lo : s * NK1 + hi, :],
                            in_=xs_d[s].rearrange("(kt p) b -> p kt b", p=128)[
                                :, lo:hi, nsl
                            ],
                        )
                else:
                    nc.sync.dma_start(
                        out=t[:],
                        in_=xs_d.rearrange("s (kt p) b -> p (s kt) b", p=128)[
                            :, :, nsl
                        ],
                    )
                xc[n] = t
                tl = xp.tile([4 * 2 * LFT, NCHUNK], fp16, tag="xlf", name=f"xlf_{n}")
                nc.sync.dma_start(out=tl[:], in_=xlf_d[:, nsl])
                xlfc[n] = tl

            load_x(0, split_dmas=True)
            mu1 = sp.tile([128, M_T], f32, tag="mu1", name="mu1")
            nc.sync.dma_start(out=mu1[:], in_=mu1_d)
            for k in range(1, NK1):
                nc.sync.dma_start(out=s1w[k][:], in_=s1t_d[k * 128 : (k + 1) * 128, :])
            s1lf = wp.tile([4 * 2 * LFT, D], fp16, tag="s1lf", name="s1lf")
            nc.sync.dma_start(out=s1lf[:], in_=s1lf_d)
            negc4 = sp.tile([16, 1], f32, tag="negc4", name="negc4")
            nc.sync.dma_start(out=negc4[:], in_=negc4_d)
            c3sb = sp.tile([128, M_T], f32, tag="c3sb", name="c3sb")
            nc.sync.dma_start(out=c3sb[:], in_=c3_d)
            warm = sp.tile([128, 1], f32, tag="warm", name="warm")
            nc.gpsimd.memset(warm[:], 1.0)
            nc.scalar.sign(warm[:], warm[:])

            # h pairs in fp8 {0,1} (DoubleRow rhs layout: plane j of pair a
            # is feature tile m = 2a + j)
            h1p = [
                hp.tile([128, 2, B_SHARD], fp8, tag=f"h1_{a}", name=f"h1_{a}")
                for a in range(A_T)
            ]
            h2p = [
                hp.tile([128, 2, B_SHARD], fp8, tag=f"h2_{a}", name=f"h2_{a}")
                for a in range(A_T)
            ]
            hacc1 = sp.tile([128, M_T * N_T], f32, tag="hacc1", name="hacc1")
            # layer-2 rowsums split by chunk parity: even chunks sign on
            # DVE in {0,1}, odd chunks on ScalarE in +-1 (both engines
            # drain in parallel after mu2 lands); col = m*2 + n//2
            hacc2a = sp.tile([128, M_T * 2], f32, tag="hacc2a", name="hacc2a")
            hacc2b = sp.tile([128, M_T * 2], f32, tag="hacc2b", name="hacc2b")

            def gt01(dst_ap, src_ap, mu_ap, accum_ap=None, eng=None):
                """h01 = (z > mu) in one op.  The comparison sits in op0
                (the verifier only restricts the op1/reduce slot, which
                stays a legal `add 0`), so accum_out can produce the {0,1}
                rowsum for the next layer's stats in the same instruction.
                eng selects VectorE (default) or GpSimd (SBUF sources only)
                so a long sign drain can run on both engines in parallel."""
                (eng or nc.vector).tensor_scalar(
                    dst_ap, src_ap, mu_ap, 0.0, op0=GT, op1=ADD,
                    accum_out=accum_ap,
                )

            # ---- layer 1 ----------------------------------------------
            # four feature tiles per group: consecutive matmuls hit
            # different PSUM banks (drains overlap), and the four K=32
            # leftover matmuls run CONCURRENTLY in 32-row strips of the
            # PE array (tile_position row tiling) -- one 512-cycle pass
            # instead of four
            for n in range(N_T):
                nsl = slice(n * NCHUNK, (n + 1) * NCHUNK)
                if n > 0:
                    load_x(n, split_dmas=False)
                for m0 in range(0, M_T, 4):
                    pss = [
                        yp.tile([128, NCHUNK], f32, tag="y", name=f"ps1_{n}_{m0 + q}")
                        for q in range(4)
                    ]
                    idx = 0
                    for s in range(2):
                        for k in range(NK1):
                            for q in range(4):
                                msl = slice((m0 + q) * 128, (m0 + q + 1) * 128)
                                nc.tensor.matmul(
                                    pss[q][:],
                                    s1w[k][:, msl],
                                    xc[n][:, s * NK1 + k, :],
                                    start=(idx == 0),
                                    stop=False,
                                )
                            idx += 1
                    for q in range(4):
                        msl = slice((m0 + q) * 128, (m0 + q + 1) * 128)
                        nc.tensor.matmul(
                            pss[q][:],
                            s1lf[32 * q : 32 * (q + 1), msl],
                            xlfc[n][32 * q : 32 * (q + 1), :],
                            start=False,
                            stop=True,
                            tile_position=(32 * q, 0),
                        )
                    for q in range(4):
                        m = m0 + q
                        c = m * N_T + n
                        gt01(
                            h1p[m // 2][:, m % 2, nsl],
                            pss[q][:],
                            mu1[:, m : m + 1],
                            hacc1[:, c : c + 1],
                        )

            # ---- later-layer weights (emitted after L1 so their DMAs don't
            # delay the x stream) ---------------------------------------
            def load_ktiles(dram_ap, tagp):
                ts = []
                for k in range(M_T):
                    t = wp.tile([128, D], bf16, tag=f"{tagp}_{k}", name=f"{tagp}{k}")
                    nc.sync.dma_start(
                        out=t[:], in_=dram_ap[k * 128 : (k + 1) * 128, :]
                    )
                    ts.append(t)
                return ts

            def load_dr(dram_ap, tagp, dout):
                ts = []
                for a in range(A_T):
                    t = wp.tile([128, 2, dout], fp8, tag=f"{tagp}_{a}", name=f"{tagp}{a}")
                    nc.sync.dma_start(out=t[:], in_=dram_ap[a])
                    ts.append(t)
                return ts

            s2b = load_ktiles(s2b_d, "s2b")
            s2dr = load_dr(s2dr_d, "s2dr", D)
            s3b = load_ktiles(s3b_d, "s3b")
            s3dr = load_dr(s3dr_d, "s3dr", D)
            s4dr = load_dr(s4dr_d, "s4dr", 16)

            if debug_outputs:
                for a in range(A_T):
                    for j in range(2):
                        nc.sync.dma_start(
                            out=dbg["dbg_h1"][:, 2 * a + j, :], in_=h1p[a][:, j, :]
                        )

            # ---- stats pipeline ---------------------------------------
            # Per-feature batch means for the next layer, computed from the
            # LOCAL {0,1} rowsums: mean(z) = (S @ rowsum01) / B, and by
            # linearity the all-reduce can run on the matvec OUTPUT -- so the
            # PE-side mean matmul needs nothing from the collective, and only
            # the is_gt bias waits for it. Everything stays exact integers.
            # Engine split: rowsum splits on GpSimd (idle, SBUF-only), PSUM
            # combine on VectorE, scale on GpSimd -- so no long-wait op sits
            # in front of later DVE/Scalar work in their FIFO queues.
            def stats(hacc, sbw, lname, do_scale=True, hacc_pm=None):
                hs = sp.tile([128, M_T], f32, tag=f"hs{lname}", name=f"hs{lname}")
                if hacc_pm is None:
                    nc.vector.reduce_sum(
                        hs[:], hacc[:].rearrange("p (m n) -> p m n", n=N_T), axis=AX
                    )
                else:
                    # mixed encodings: hacc holds {0,1} rowsums (2 even
                    # chunks), hacc_pm holds +-1 rowsums (2 odd chunks);
                    # rowsum01 = sum01 + (sum_pm + 2*NCHUNK) / 2, exact ints
                    ha = sp.tile([128, M_T], f32, tag=f"hsa{lname}", name=f"hsa{lname}")
                    hb = sp.tile([128, M_T], f32, tag=f"hsb{lname}", name=f"hsb{lname}")
                    nc.vector.reduce_sum(
                        ha[:], hacc[:].rearrange("p (m n) -> p m n", n=2), axis=AX
                    )
                    nc.vector.reduce_sum(
                        hb[:], hacc_pm[:].rearrange("p (m n) -> p m n", n=2), axis=AX
                    )
                    nc.vector.tensor_scalar(
                        hb[:], hb[:], float(2 * NCHUNK), 0.5, op0=ADD, op1=MUL
                    )
                    nc.vector.tensor_add(hs[:], ha[:], hb[:])
                # exact split of local integer rowsums (<= 2048) into two
                # bf16 halves for the mean matmul
                hh = sp.tile([128, M_T, 2], bf16, tag=f"hh{lname}", name=f"hh{lname}")
                hhif = sp.tile(
                    [128, M_T], f32, tag=f"hhif{lname}", name=f"hhif{lname}"
                )
                nc.gpsimd.tensor_copy(hh[:, :, 0], hs[:])
                nc.gpsimd.tensor_copy(hhif[:], hh[:, :, 0])
                nc.gpsimd.tensor_sub(hh[:, :, 1], hs[:], hhif[:])
                pmu = mp.tile([128, M_T, 2], f32, tag="mu", name=f"pmu{lname}")
                for m in range(M_T):
                    msl = slice(m * 128, (m + 1) * 128)
                    for k in range(M_T):
                        nc.tensor.matmul(
                            pmu[:, m, :],
                            sbw[k][:, msl],
                            hh[:, k, :],
                            start=(k == 0),
                            stop=(k == M_T - 1),
                        )
                pml = sp.tile([128, M_T], f32, tag=f"pml{lname}", name=f"pml{lname}")
                # pairwise add of the bf16-half columns == reduce over the
                # innermost axis (one PSUM input; two would be rejected)
                nc.vector.reduce_sum(pml[:], pmu[:], axis=AX)
                ar_in = dp.tile([128, M_T], f32, tag=f"ari{lname}", name=f"ari{lname}")
                ar_out = dp.tile([128, M_T], f32, tag=f"aro{lname}", name=f"aro{lname}")
                nc.sync.dma_start(out=ar_in[:], in_=pml[:])
                nc.gpsimd.collective_compute(
                    "AllReduce",
                    mybir.AluOpType.add,
                    replica_groups=[list(range(N_CORES))],
                    ins=[ar_in[:].opt()],
                    outs=[ar_out[:].opt()],
                )
                asb = sp.tile([128, M_T], f32, tag=f"asb{lname}", name=f"asb{lname}")
                nc.sync.dma_start(out=asb[:], in_=ar_out[:])
                if not do_scale:
                    # consumer folds the 1/B scale into its compare
                    return asb
                mu = sp.tile([128, M_T], f32, tag=f"mu{lname}", name=f"mu{lname}")
                nc.vector.tensor_scalar_mul(mu[:], asb[:], 1.0 / B_FULL)
                return mu

            # ---- layer 2 (fp8 DoubleRow, n-major; first 30 chunks staged
            # to fp16 so the PE never waits for the stats round-trip) ----
            def lx_mms_pair(ldr, h_in, n, m0, tagp):
                """Two DoubleRow chunks (m0, m0+1) with interleaved groups."""
                nsl = slice(n * NCHUNK, (n + 1) * NCHUNK)
                pss = [
                    yp.tile([128, NCHUNK], f32, tag="y", name=f"ps{tagp}_{n}_{m0 + j}")
                    for j in range(2)
                ]
                for a in range(A_T):
                    for j in range(2):
                        msl = slice((m0 + j) * 128, (m0 + j + 1) * 128)
                        nc.tensor.matmul(
                            pss[j][:],
                            ldr[a][:, :, msl],
                            h_in[a][:, :, nsl],
                            start=(a == 0),
                            stop=(a == A_T - 1),
                            perf_mode=DR,
                        )
                return pss

            def sign2(src_ap, n, m):
                """Even chunks: {0,1} on DVE; odd chunks: +-1 via ScalarE
                Sign -- both engines drain in parallel after mu2 lands.
                Downstream, odd-chunk z3 columns are true +-1-encoded values
                (compared against negmu3pm) while even-chunk columns stay in
                the {0,1} algebra (compared against M3); exact either way."""
                nsl = slice(n * NCHUNK, (n + 1) * NCHUNK)
                dst = h2p[m // 2][:, m % 2, nsl]
                c = m * 2 + n // 2
                if n % 2 == 0:
                    gt01(dst, src_ap, mu2[:, m : m + 1], hacc2a[:, c : c + 1])
                else:
                    nc.scalar.activation(
                        dst, src_ap, AF_SIGN,
                        bias=negmu2[:, m : m + 1],
                        accum_out=hacc2b[:, c : c + 1],
                    )

            pairs = [(n, m0) for n in range(N_T) for m0 in range(0, M_T, 2)]
            N_STG2P = 16  # stage ALL chunks: every sign2 src is SBUF, so
            # the post-collective drain can split across DVE and GpSimd
            staged2 = []

            def stage2(n, m0):
                pss = lx_mms_pair(s2dr, h1p, n, m0, "2")
                for j in range(2):
                    st = stp.tile(
                        [128, NCHUNK], fp16, tag="stg", name=f"st2_{n}_{m0 + j}"
                    )
                    # all stage copies on ScalarE: it is otherwise idle, and
                    # keeping them off VectorE means the sign drain and the
                    # stats fixups never block a copy in the DVE FIFO
                    nc.scalar.copy(st[:], pss[j][:])
                    staged2.append((st, n, m0 + j))

            # hacc1 completes with layer 1's last sign, so the stats matvec
            # is ready almost immediately -- emit it one pair in
            stage2(*pairs[0])
            M2 = stats(hacc1, s2b, "1", do_scale=False)
            # exact dyadic scales: mu2 = M2 / 2^14 (for the DVE is_gt) and
            # -mu2 (bias for the ScalarE Sign path); gpsimd is idle here
            mu2 = sp.tile([128, M_T], f32, tag="mu2s", name="mu2s")
            nc.vector.tensor_scalar_mul(mu2[:], M2[:], 1.0 / B_FULL)
            negmu2 = sp.tile([128, M_T], f32, tag="negmu2", name="negmu2")
            nc.gpsimd.tensor_scalar_mul(negmu2[:], M2[:], -(1.0 / B_FULL))
            for n, m0 in pairs[1:N_STG2P]:
                stage2(n, m0)
            # drain chunks 0/1 first (DVE and ScalarE in parallel), so
            # L3's first matmuls unblock fastest; chunks 2/3 are emitted
            # after stage3(pair 0) so that pair's PSUM->SBUF copies are
            # not head-of-line blocked behind the whole Scalar sign queue
            for st, n, m in staged2:
                if n <= 1:
                    sign2(st[:], n, m)

            if debug_outputs:
                for a in range(A_T):
                    for j in range(2):
                        nc.sync.dma_start(
                            out=dbg["dbg_h2"][:, 2 * a + j, :], in_=h2p[a][:, j, :]
                        )

            # ---- layer 3 + layer 4 ------------------------------------
            h3c = {}  # n -> list of pair tiles

            def h3_tile(n, m):
                a = m // 2
                if n not in h3c:
                    h3c[n] = [None] * A_T
                if h3c[n][a] is None:
                    h3c[n][a] = h3p_pool.tile(
                        [128, 2, NCHUNK], fp8, tag=f"h3_{a}", name=f"h3_{a}_{n}"
                    )
                return h3c[n][a]

            def sign3(src_ap, n, m):
                # even chunks: {0,1} on VectorE (compare B*z > M3, exact);
                # odd chunks: +-1 via ScalarE Sign(z - mu3) -- both engines
                # drain in parallel, halving the serial tail
                t = h3_tile(n, m)
                if n % 2 == 0:
                    nc.vector.tensor_scalar(
                        t[:, m % 2, :], src_ap, float(B_FULL),
                        M3[:, m : m + 1], op0=MUL, op1=GT,
                    )
                else:
                    nc.scalar.activation(
                        t[:, m % 2, :], src_ap, AF_SIGN,
                        bias=negmu3[:, m : m + 1],
                    )

            def l4(n):
                nsl = slice(n * NCHUNK, (n + 1) * NCHUNK)
                p4 = mp.tile([16, NCHUNK], f32, tag="mu", name=f"p4_{n}")
                for a in range(A_T):
                    nc.tensor.matmul(
                        p4[:],
                        s4dr[a][:],
                        h3c[n][a][:],
                        start=(a == 0),
                        stop=(a == A_T - 1),
                        perf_mode=DR,
                    )
                oc = op_.tile([D_OUT, NCHUNK], f32, tag="oc", name=f"oc_{n}")
                if n % 2 == 0:
                    # {0,1}-encoded h3: out = 2*z4 - rowsum(S4)
                    nc.scalar.activation(
                        oc[:],
                        p4[0:D_OUT, :],
                        AF_IDENT,
                        bias=negc4[0:D_OUT, :],
                        scale=2.0,
                    )
                else:
                    # +-1-encoded h3: z4 is already the exact output
                    nc.vector.tensor_copy(oc[:], p4[0:D_OUT, :])
                nc.sync.dma_start(out=out_d[:, nsl], in_=oc[:])

            N_STG3P = 13  # the last 3 pairs hold their PSUM banks until
            # mu3 lands and sign directly from PSUM (saves 6 tail copies)
            staged3 = []

            def stage3(n, m0):
                pss = lx_mms_pair(s3dr, h2p, n, m0, "3")
                for j in range(2):
                    st = stp.tile(
                        [128, NCHUNK], fp16, tag="stg", name=f"st3_{n}_{m0 + j}"
                    )
                    nc.scalar.copy(st[:], pss[j][:])
                    staged3.append((st, n, m0 + j))

            # stats2's matvec depends on the full sign2 drain; emit it one
            # pair in so the collective launches as early as possible
            stage3(*pairs[0])
            for st, n, m in staged2:
                if n >= 2:
                    sign2(st[:], n, m)
            M3 = stats(hacc2a, s3b, "2", do_scale=False, hacc_pm=hacc2b)
            # odd chunks carry true +-1-encoded z3 (their h2 is +-1), whose
            # batch mean is mu3_pm = 2*mu3_01 - c3 with c3 = rowsum(S3);
            # bias for the Sign path: negmu3pm = c3 - (2/B)*M3, exact in
            # fp32 (25-bit window proof: |mu3_pm| < 2^11, LSB 2^-13)
            negmu3 = sp.tile([128, M_T], f32, tag="negmu3", name="negmu3")
            nc.vector.tensor_scalar_mul(negmu3[:], M3[:], -(2.0 / B_FULL))
            nc.vector.tensor_add(negmu3[:], negmu3[:], c3sb[:])
            for n, m0 in pairs[1:N_STG3P]:
                stage3(n, m0)
            # drain the staged chunks n-major, launching each chunk's L4 as
            # soon as its 8 feature tiles are signed, so the output tail
            # overlaps the drain instead of serializing after it
            done_m = {n: 0 for n in range(N_T)}

            def after_sign(n):
                done_m[n] += 1
                if done_m[n] == M_T:
                    l4(n)

            for st, n, m in staged3:
                sign3(st[:], n, m)
                after_sign(n)
            for n, m0 in pairs[N_STG3P:]:
                pss = lx_mms_pair(s3dr, h2p, n, m0, "3")
                for j in range(2):
                    sign3(pss[j][:], n, m0 + j)
                    after_sign(n)

    nc.compile()
    return nc


def _get_program():
    global _PROGRAM
    if _PROGRAM is None:
        _PROGRAM = _build_program()
    return _PROGRAM


def _split2_fp16(a32):
    """Split fp32 array into two fp16 terms summing to a32 (to ~2^-22 rel)."""
    a0 = a32.astype(FP16)
    r = a32 - a0.astype(np.float32)
    a1 = r.astype(FP16)
    return a0, a1


def _dr_layout(st, dout):
    """[D, dout] K-major sign matrix -> DoubleRow lhsT blocks
    [A_T, 128, 2, dout] with element (a, ki, j, q) = st[(2a+j)*128 + ki, q]."""
    return np.ascontiguousarray(
        st.reshape(A_T, 2, 128, dout).transpose(0, 2, 1, 3).astype(FP8)
    )


def _numpy_fallback(x, W1, g1, b1, W2, g2, b2, W3, g3, b3, W4):
    eps = np.float32(1e-5)

    def bn_sign(y, g, b):
        mu = y.mean(axis=0, dtype=np.float32)
        var = np.mean(np.square(y - mu), axis=0, dtype=np.float32)
        return np.sign(g * (y - mu) / np.sqrt(var + eps) + b).astype(np.float32)

    h = bn_sign(x @ np.sign(W1).T, g1, b1)
    h = bn_sign(h @ np.sign(W2).T, g2, b2)
    h = bn_sign(h @ np.sign(W3).T, g3, b3)
    return (h @ np.sign(W4).T).astype(np.float32)


def kernel(x, W1, g1, b1, W2, g2, b2, W3, g3, b3, W4):
    global LAST_RESULTS
    x = np.asarray(x, np.float32).reshape(-1, D_IN)
    args = [np.asarray(a, np.float32) for a in (W1, g1, b1, W2, g2, b2, W3, g3, b3, W4)]
    W1, g1, b1, W2, g2, b2, W3, g3, b3, W4 = args

    specializable = (
        x.shape == (B_FULL, D_IN)
        and all((g > 0).all() for g in (g1, g2, g3))
        and all((b == 0).all() for b in (b1, b2, b3))
    )
    if not specializable:
        return _numpy_fallback(x, W1, g1, b1, W2, g2, b2, W3, g3, b3, W4)

    from concourse.bass_utils import run_bass_kernel_spmd

    s1 = np.sign(W1)  # [1024, 784]
    s1t_full = np.ascontiguousarray(s1.T.astype(FP16))  # [784, 1024]
    s1t = s1t_full[: NK1 * 128]
    # both split planes' leftover rows, replicated 4x along partitions
    # for the tile_position row-tiled leftover matmuls
    s1lf = np.tile(
        np.concatenate([s1t_full[NK1 * 128 :]] * 2, axis=0), (4, 1)
    )  # [128, 1024]
    s2t = np.ascontiguousarray(np.sign(W2).T)  # [in, out] f32
    s3t = np.ascontiguousarray(np.sign(W3).T)
    s4t = np.ascontiguousarray(np.sign(W4).T)

    xt = np.ascontiguousarray(x.T) * np.float32(XSCALE)  # [784, 16384]
    x0, x1 = _split2_fp16(xt)
    xs_full = np.stack([x0[: NK1 * 128], x1[: NK1 * 128]])  # [2, 768, B]
    xlf_full = np.tile(
        np.concatenate([x0[NK1 * 128 :], x1[NK1 * 128 :]], axis=0), (4, 1)
    )  # [128, B]

    # layer-1 batch mean, computed on host in fp64 (scaled by XSCALE):
    # mean(x @ S1.T, axis=0) == (S1 @ sum(x, axis=0)) / B
    xsum = x.sum(axis=0, dtype=np.float64)  # [784]
    mu1 = (s1.astype(np.float64) @ xsum) * (XSCALE / float(B_FULL))  # [1024]
    mu1_in = np.ascontiguousarray(
        mu1.astype(np.float32).reshape(M_T, 128).T
    )  # [128, M_T], column m <-> features m*128 + p
    # c3 = per-feature rowsum of S3, for the +-1-encoded odd-chunk bias
    c3_in = np.ascontiguousarray(
        s3t.sum(axis=0, dtype=np.float64).astype(np.float32).reshape(M_T, 128).T
    )  # [128, M_T]
    # out = 2*z4 - rowsum(S4) fixup for the {0,1} encoding
    negc4 = np.zeros((16, 1), np.float32)
    negc4[:D_OUT, 0] = -s4t.sum(axis=0, dtype=np.float64).astype(np.float32)

    common = {
        "s1t": s1t,
        "s1lf": np.ascontiguousarray(s1lf),
        "s2b": np.ascontiguousarray(s2t.astype(BF16)),
        "s3b": np.ascontiguousarray(s3t.astype(BF16)),
        "s2dr": _dr_layout(s2t, D),
        "s3dr": _dr_layout(s3t, D),
        "s4dr": _dr_layout(np.concatenate([s4t, np.zeros((D, 6), s4t.dtype)], axis=1), 16),
        "mu1": mu1_in,
        "c3": c3_in,
        "negc4": negc4,
    }
    in_maps = []
    for c in range(N_CORES):
        sl = slice(c * B_SHARD, (c + 1) * B_SHARD)
        in_maps.append(
            {
                "xs": np.ascontiguousarray(xs_full[:, :, sl]),
                "xlf": np.ascontiguousarray(xlf_full[:, sl]),
                **common,
            }
        )

    nc = _get_program()
    LAST_RESULTS = run_bass_kernel_spmd(nc, in_maps, core_ids=list(range(N_CORES)))
    y = np.concatenate(
        [LAST_RESULTS.results[c]["out"] for c in range(N_CORES)], axis=1
    )  # [10, 16384]
    return np.ascontiguousarray(y.T).astype(np.float32)

